# revision 19
# baseline (speedup 1.0000x reference)
"""Trainium2 Bass kernel for the batched differentiable EKF.

Problem shape (hardcoded): B=2048, T=200, S=16 state dim, M=8 meas dim.

Structure exploited:
  * The covariance recursion (P_pred, S, K, P_new) never depends on the
    measurements — only on A, C, Q, R and the initial covariance.
  * The provided initial_covariance is identical for every batch element
    (checked at runtime), so the whole P/K track is batch-constant and is
    computed once on host in float64 (200 steps of 16x16 ops).
  * What remains per batch element is a linear time-varying recurrence
        x_t = x_{t-1} @ M_t + z_t @ N_t,   zpred_t = x_{t-1} @ G
    which is evaluated on device in chunks of 16 steps: each chunk is a
    single pair of accumulating matmuls against precomputed chunk-transfer
    weights, so the serial dependency chain is only 13 hops long.
  * The covariances output [B,T,S,S] (419 MB) is a broadcast of the
    batch-constant P track; the device replicates it to DRAM with wide
    SBUF->DRAM DMAs (8 batch rows = 1.6 MB per DMA).

Sharding: pure data parallel over batch, 256 rows per core, 8 cores.
"""

import numpy as np

_EPS = 1e-6      # numerical_stability_eps
_MIN_EIG = 1e-6  # LearnableCovariance.min_eigenvalue

B, T, S, M = 2048, 200, 16, 8
NCORES = 8
BS = B // NCORES          # 256 batch rows per core
CH = 14                   # chunk length (16 + 8*CH = 128 = exact K tile)
NCH = (T + CH - 1) // CH  # 15 chunks
LAST = T - CH * (NCH - 1) # 4 steps in the last chunk
WCOLS = 24 * T            # 4800 total weight columns (16+8 outputs/step)

_CACHE = {}
LAST_RESULT = None  # BassKernelResults of the most recent device run


# ---------------------------------------------------------------- host math

def _learned_cov(log_diag, off_diag, n):
    d = np.maximum(np.exp(np.asarray(log_diag, np.float64)), _MIN_EIG)
    L = np.diag(d)
    r, c = np.tril_indices(n, -1)
    if len(r) > 0:
        L[r, c] = np.asarray(off_diag, np.float64)
    return L @ L.T


def _reg(P):
    P = 0.5 * (P + P.T)
    return P + _EPS * np.eye(P.shape[-1], dtype=P.dtype)


def _p_track(A, C, Q, R, P0):
    """Batch-constant covariance recursion. Returns per-step state transfer
    M_t [S,S], measurement gain N_t [M,S], stored covariance P_t [S,S]."""
    A = np.asarray(A, np.float64)
    C = np.asarray(C, np.float64)
    P = np.asarray(P0, np.float64)
    I = np.eye(S)
    Ms = np.empty((T, S, S))
    Ns = np.empty((T, M, S))
    Pseq = np.empty((T, S, S))
    for t in range(T):
        Pp = _reg(A @ P @ A.T + Q)
        Sm = _reg(C @ Pp @ C.T + R)
        K = Pp @ C.T @ np.linalg.inv(Sm)
        IKH = I - K @ C
        Pn = _reg(IKH @ Pp @ IKH.T + K @ R @ K.T)
        Ms[t] = A.T @ IKH.T
        Ns[t] = K.T
        Pseq[t] = Pn
        P = Pn
    G = A.T @ C.T  # zpred_t = x_{t-1} @ G
    return Ms, Ns, Pseq, G


def _chunk_weights(Ms, Ns, G):
    """Per-chunk transfer weights mapping u = [x_chunk_start; z_1..z_c]
    (as rows) to [states(16c) | zpreds(8c)] (as columns).  Returned as a
    single [16+8*CH, WCOLS] array; chunk ch occupies columns
    24*CH*ch .. +24c, with rows 0..15 the x part and 16..16+8c the z part."""
    U = np.zeros((S + 8 * CH, WCOLS))
    for ch in range(NCH):
        c = CH if ch < NCH - 1 else LAST
        col0 = 24 * CH * ch
        V = np.zeros((S + 8 * c, S))
        V[:S, :S] = np.eye(S)
        for j in range(1, c + 1):
            t = CH * ch + j  # 1-indexed global step
            zp = V @ G       # zpred_t = X_{t-1} @ G
            czp = col0 + 16 * c + 8 * (j - 1)
            U[:S + 8 * c, czp:czp + 8] = zp
            V = V @ Ms[t - 1]
            V[S + 8 * (j - 1):S + 8 * j, :] += Ns[t - 1]
            cst = col0 + 16 * (j - 1)
            U[:S + 8 * c, cst:cst + 16] = V
    return np.ascontiguousarray(U, np.float32)


# ------------------------------------------------------------ device kernel

def _fix_drain_waits(nc, mybir):
    """This walrus build rejects instructions carrying more semaphore waits
    than their ctrl struct holds ("Too many sync wait commands") — seen on
    InstDrain and on matmul (waits migrate to LDWEIGHTS).  Cap inline waits
    (0 for Drain/Matmult, 1 otherwise); hoist the rest onto NoOps."""
    ctr = 0
    for f in nc.m.functions:
        for bb in f.blocks:
            new_insts = []
            for inst in bb.instructions:
                si = getattr(inst, "sync_info", None)
                cap = 1
                if isinstance(inst, mybir.InstDrain):
                    cap = 0
                if si is not None and si.on_wait and len(si.on_wait) > cap:
                    for w in si.on_wait[cap:]:
                        ctr += 1
                        new_insts.append(mybir.InstNoOp(
                            name=f"I-waitfix-{ctr}",
                            engine=inst.engine,
                            sync_info=mybir.SyncInfo(on_wait=[w], on_update=[]),
                            bass_nofuse=True,
                        ))
                    si.on_wait = si.on_wait[:cap]
                new_insts.append(inst)
            bb.instructions[:] = new_insts


def _build_bass():
    import concourse.bass as bass
    import concourse.mybir as mybir
    import concourse.tile as tile

    f32 = mybir.dt.float32
    nc = bass.Bass()
    x0T_d = nc.dram_tensor("x0T", [S, BS], f32, kind="ExternalInput")
    measT_d = nc.dram_tensor("measT", [8 * T, BS], f32, kind="ExternalInput")
    U_d = nc.dram_tensor("U", [S + 8 * CH, WCOLS], f32, kind="ExternalInput")
    Prep_d = nc.dram_tensor("Prep", [2, 128, 1600], f32, kind="ExternalInput")
    st_out = nc.dram_tensor("st", [2, 128, 3200], f32, kind="ExternalOutput")
    zp_out = nc.dram_tensor("zp", [2, 128, 1600], f32, kind="ExternalOutput")
    cv_out = nc.dram_tensor("cv", [BS // 8, 128, 3200], f32, kind="ExternalOutput")

    with tile.TileContext(nc) as tc:
        with tc.tile_pool(name="const", bufs=1) as cpool, \
             tc.tile_pool(name="stage", bufs=1) as spool, \
             tc.tile_pool(name="u", bufs=3) as upool, \
             tc.tile_pool(name="py", bufs=4, space="PSUM") as pypool, \
             tc.tile_pool(name="px", bufs=2, space="PSUM") as pxpool:

            # P-track source in two halves; first on the low-latency ACT
            # HWDGE ring so broadcast writes start ~2us in
            prs = [cpool.tile([128, 1600], f32, name=f"pr{i}") for i in (0, 1)]
            nc.scalar.dma_start(prs[0][:], Prep_d[0])
            nc.gpsimd.dma_start(prs[1][:], Prep_d[1])
            U = cpool.tile([S + 8 * CH, WCOLS], f32)
            nc.gpsimd.dma_start(U[:], U_d[:])

            # covariance broadcast: 2 DMAs per 8 batch rows (0.8 MB each)
            for i in range(BS // 8):
                nc.sync.dma_start(cv_out[i, :, 0:1600], prs[0][:])
                nc.sync.dma_start(cv_out[i, :, 1600:3200], prs[1][:])

            sts = [spool.tile([128, 3200], f32, tag=f"st{h}", name=f"st{h}")
                   for h in (0, 1)]
            zps = [spool.tile([128, 1600], f32, tag=f"zp{h}", name=f"zp{h}")
                   for h in (0, 1)]

            # chunk-0 input tiles: rows 0..15 = x0, rows 16.. = measurements
            u_cur = []
            for h in (0, 1):
                u0 = upool.tile([128, 128], f32, tag=f"u{h}", name=f"u0_{h}")
                nc.gpsimd.dma_start(u0[0:S, :], x0T_d[:, 128 * h:128 * h + 128])
                nc.gpsimd.dma_start(u0[S:S + 8 * CH, :],
                                    measT_d[0:8 * CH, 128 * h:128 * h + 128])
                u_cur.append(u0)

            for ch in range(NCH):
                c = CH if ch < NCH - 1 else LAST
                col0, kz = 24 * CH * ch, 8 * c
                K = S + kz
                for h in (0, 1):
                    u = u_cur[h]
                    py = pypool.tile([128, 24 * c], f32, tag="py", name=f"py{ch}_{h}")
                    nc.tensor.matmul(py[:], u[:K, :], U[:K, col0:col0 + 24 * c],
                                     start=True, stop=True)
                    if ch < NCH - 1:
                        c2 = CH if ch + 1 < NCH - 1 else LAST
                        un = upool.tile([128, 128], f32, tag=f"u{h}",
                                        name=f"u{ch + 1}_{h}")
                        nc.gpsimd.dma_start(
                            un[S:S + 8 * c2, :],
                            measT_d[8 * CH * (ch + 1):8 * CH * (ch + 1) + 8 * c2,
                                    128 * h:128 * h + 128])
                        ecol = col0 + 16 * c - 16
                        px = pxpool.tile([S, 128], f32, tag="px", name=f"px{ch}_{h}")
                        nc.tensor.matmul(px[:], U[:K, ecol:ecol + 16], u[:K, :],
                                         start=True, stop=True)
                        nc.vector.tensor_copy(un[0:S, :], px[:])
                        u_cur[h] = un
                    nc.vector.tensor_copy(sts[h][:, 16 * CH * ch:16 * CH * ch + 16 * c],
                                          py[:, :16 * c])
                    nc.vector.tensor_copy(zps[h][:, 8 * CH * ch:8 * CH * ch + 8 * c],
                                          py[:, 16 * c:24 * c])

            for h in (0, 1):
                nc.scalar.dma_start(st_out[h], sts[h][:])
                nc.scalar.dma_start(zp_out[h], zps[h][:])

    _fix_drain_waits(nc, mybir)
    return nc


def _ensure_axon_hooks():
    """This image's `antenv` lacks `axon_hooks`, which run_bass_kernel_spmd
    imports unconditionally when tracing is requested.  Provide a shim; if
    the libaxon profiling symbols exist, wire up the real ctypes hook."""
    import sys
    import types
    try:
        import antenv.axon_hooks  # noqa: F401
        return
    except ImportError:
        pass
    mod = types.ModuleType("antenv.axon_hooks")
    holder = [None]
    mod.set_axon_ntff_profile_hook = lambda h: holder.__setitem__(0, h)
    mod.get_axon_ntff_profile_hook = lambda: holder[0]
    sys.modules["antenv.axon_hooks"] = mod
    try:
        import antenv
        antenv.axon_hooks = mod
    except ImportError:
        pass
    try:
        from trn_agent_boot.trn_boot import _ntff_profile_via_ctypes
        hook = _ntff_profile_via_ctypes("/opt/axon/libaxon_pjrt.so")
        if hook is not None:
            mod.set_axon_ntff_profile_hook(hook)
    except Exception:
        pass


# ------------------------------------------------------------ numpy fallback

def _fallback(initial_state, initial_covariance, measurements, A, C,
              log_diag_Q, offdiag_Q, log_diag_R, offdiag_R):
    """Full per-batch EKF on host (used only if initial_covariance is not
    batch-constant, which setup_inputs never produces)."""
    Q = _learned_cov(log_diag_Q, offdiag_Q, S)
    R = _learned_cov(log_diag_R, offdiag_R, M)
    A = np.asarray(A, np.float64)
    C = np.asarray(C, np.float64)
    x = np.asarray(initial_state, np.float64)
    P = np.asarray(initial_covariance, np.float64)
    z = np.asarray(measurements, np.float64)
    I = np.eye(S)

    def regb(Pb):
        Pb = 0.5 * (Pb + np.swapaxes(Pb, -1, -2))
        return Pb + _EPS * I

    xs = np.empty((B, T, S), np.float32)
    Ps = np.empty((B, T, S, S), np.float32)
    zs = np.empty((B, T, M), np.float32)
    for t in range(T):
        x = x @ A.T
        P = regb(np.einsum('ij,bjk,lk->bil', A, P, A) + Q)
        zp = x @ C.T
        Sm = regb(np.einsum('ij,bjk,lk->bil', C, P, C) + R)
        Kg = np.einsum('bij,kj->bik', P, C) @ np.linalg.inv(Sm)
        x = x + np.einsum('bsm,bm->bs', Kg, z[:, t] - zp)
        IKH = I - Kg @ C
        P = regb(np.einsum('bij,bjk,blk->bil', IKH, P, IKH)
                 + np.einsum('bim,mn,bjn->bij', Kg, R, Kg))
        xs[:, t] = x
        Ps[:, t] = P
        zs[:, t] = zp
    return xs, Ps, zs


# ------------------------------------------------------------------- kernel

def kernel(initial_state, initial_covariance, measurements, A, C,
           log_diag_Q, offdiag_Q, log_diag_R, offdiag_R):
    global LAST_RESULT
    initial_state = np.asarray(initial_state, np.float32)
    initial_covariance = np.asarray(initial_covariance, np.float32)
    measurements = np.asarray(measurements, np.float32)

    P0 = initial_covariance[0]
    if initial_covariance.shape != (B, S, S) or \
       not (initial_covariance == P0[None]).all():
        return _fallback(initial_state, initial_covariance, measurements, A, C,
                         log_diag_Q, offdiag_Q, log_diag_R, offdiag_R)

    Q = _learned_cov(log_diag_Q, offdiag_Q, S)
    R = _learned_cov(log_diag_R, offdiag_R, M)
    Ms, Ns, Pseq, G = _p_track(A, C, Q, R, P0)
    U = _chunk_weights(Ms, Ns, G)
    Pflat = np.ascontiguousarray(Pseq, np.float32).reshape(T * S * S)
    Prep128 = np.tile(Pflat.reshape(16, 3200), (8, 1))   # [128, 3200]
    Prep = np.ascontiguousarray(
        np.stack([Prep128[:, :1600], Prep128[:, 1600:]]))  # [2, 128, 1600]

    x0T = np.ascontiguousarray(initial_state.T)                     # [S, B]
    measT = np.ascontiguousarray(measurements.reshape(B, 8 * T).T)  # [8T, B]

    if "nc" not in _CACHE:
        _CACHE["nc"] = _build_bass()
    nc = _CACHE["nc"]

    _ensure_axon_hooks()
    from concourse.bass_utils import run_bass_kernel_spmd
    in_maps = []
    for k in range(NCORES):
        bsl = slice(k * BS, (k + 1) * BS)
        in_maps.append({
            "x0T": np.ascontiguousarray(x0T[:, bsl]),
            "measT": np.ascontiguousarray(measT[:, bsl]),
            "U": U, "Prep": Prep,
        })
    res = run_bass_kernel_spmd(nc, in_maps, list(range(NCORES)))
    LAST_RESULT = res

    states = np.empty((B, T, S), np.float32)
    covs = np.empty((B, T, S, S), np.float32)
    zpred = np.empty((B, T, M), np.float32)
    for k in range(NCORES):
        bsl = slice(k * BS, (k + 1) * BS)
        r = res.results[k]
        states[bsl] = r["st"].reshape(BS, T, S)
        covs[bsl] = r["cv"].reshape(BS, T, S, S)
        zpred[bsl] = r["zp"].reshape(BS, T, M)
    return states, covs, zpred


# revision 20
# speedup vs baseline: 1.0123x; 1.0123x over previous
"""Trainium2 Bass kernel for the batched differentiable EKF.

Problem shape (hardcoded): B=2048, T=200, S=16 state dim, M=8 meas dim.

Structure exploited:
  * The covariance recursion (P_pred, S, K, P_new) never depends on the
    measurements — only on A, C, Q, R and the initial covariance.
  * The provided initial_covariance is identical for every batch element
    (checked at runtime), so the whole P/K track is batch-constant and is
    computed once on host in float64 (200 steps of 16x16 ops).
  * What remains per batch element is a linear time-varying recurrence
        x_t = x_{t-1} @ M_t + z_t @ N_t,   zpred_t = x_{t-1} @ G
    which is evaluated on device in chunks of 16 steps: each chunk is a
    single pair of accumulating matmuls against precomputed chunk-transfer
    weights, so the serial dependency chain is only 13 hops long.
  * The covariances output [B,T,S,S] (419 MB) is a broadcast of the
    batch-constant P track; the device replicates it to DRAM with wide
    SBUF->DRAM DMAs (8 batch rows = 1.6 MB per DMA).

Sharding: pure data parallel over batch, 256 rows per core, 8 cores.
"""

import numpy as np

_EPS = 1e-6      # numerical_stability_eps
_MIN_EIG = 1e-6  # LearnableCovariance.min_eigenvalue

B, T, S, M = 2048, 200, 16, 8
NCORES = 8
BS = B // NCORES          # 256 batch rows per core
CH = 14                   # chunk length (16 + 8*CH = 128 = exact K tile)
NCH = (T + CH - 1) // CH  # 15 chunks
LAST = T - CH * (NCH - 1) # 4 steps in the last chunk
WCOLS = 24 * T            # 4800 total weight columns (16+8 outputs/step)

_CACHE = {}
LAST_RESULT = None  # BassKernelResults of the most recent device run


# ---------------------------------------------------------------- host math

def _learned_cov(log_diag, off_diag, n):
    d = np.maximum(np.exp(np.asarray(log_diag, np.float64)), _MIN_EIG)
    L = np.diag(d)
    r, c = np.tril_indices(n, -1)
    if len(r) > 0:
        L[r, c] = np.asarray(off_diag, np.float64)
    return L @ L.T


def _reg(P):
    P = 0.5 * (P + P.T)
    return P + _EPS * np.eye(P.shape[-1], dtype=P.dtype)


def _p_track(A, C, Q, R, P0):
    """Batch-constant covariance recursion. Returns per-step state transfer
    M_t [S,S], measurement gain N_t [M,S], stored covariance P_t [S,S]."""
    A = np.asarray(A, np.float64)
    C = np.asarray(C, np.float64)
    P = np.asarray(P0, np.float64)
    I = np.eye(S)
    Ms = np.empty((T, S, S))
    Ns = np.empty((T, M, S))
    Pseq = np.empty((T, S, S))
    for t in range(T):
        Pp = _reg(A @ P @ A.T + Q)
        Sm = _reg(C @ Pp @ C.T + R)
        K = Pp @ C.T @ np.linalg.inv(Sm)
        IKH = I - K @ C
        Pn = _reg(IKH @ Pp @ IKH.T + K @ R @ K.T)
        Ms[t] = A.T @ IKH.T
        Ns[t] = K.T
        Pseq[t] = Pn
        P = Pn
    G = A.T @ C.T  # zpred_t = x_{t-1} @ G
    return Ms, Ns, Pseq, G


def _chunk_weights(Ms, Ns, G):
    """Per-chunk transfer weights mapping u = [x_chunk_start; z_1..z_c]
    (as rows) to [states(16c) | zpreds(8c)] (as columns).  Returned as a
    single [16+8*CH, WCOLS] array; chunk ch occupies columns
    24*CH*ch .. +24c, with rows 0..15 the x part and 16..16+8c the z part."""
    U = np.zeros((S + 8 * CH, WCOLS))
    for ch in range(NCH):
        c = CH if ch < NCH - 1 else LAST
        col0 = 24 * CH * ch
        V = np.zeros((S + 8 * c, S))
        V[:S, :S] = np.eye(S)
        for j in range(1, c + 1):
            t = CH * ch + j  # 1-indexed global step
            zp = V @ G       # zpred_t = X_{t-1} @ G
            czp = col0 + 16 * c + 8 * (j - 1)
            U[:S + 8 * c, czp:czp + 8] = zp
            V = V @ Ms[t - 1]
            V[S + 8 * (j - 1):S + 8 * j, :] += Ns[t - 1]
            cst = col0 + 16 * (j - 1)
            U[:S + 8 * c, cst:cst + 16] = V
    return np.ascontiguousarray(U, np.float32)


# ------------------------------------------------------------ device kernel

def _fix_drain_waits(nc, mybir):
    """This walrus build rejects instructions carrying more semaphore waits
    than their ctrl struct holds ("Too many sync wait commands") — seen on
    InstDrain and on matmul (waits migrate to LDWEIGHTS).  Cap inline waits
    (0 for Drain/Matmult, 1 otherwise); hoist the rest onto NoOps."""
    ctr = 0
    for f in nc.m.functions:
        for bb in f.blocks:
            new_insts = []
            for inst in bb.instructions:
                si = getattr(inst, "sync_info", None)
                cap = 1
                if isinstance(inst, (mybir.InstDrain, mybir.InstMatmult)):
                    cap = 0
                if si is not None and si.on_wait and len(si.on_wait) > cap:
                    for w in si.on_wait[cap:]:
                        ctr += 1
                        new_insts.append(mybir.InstNoOp(
                            name=f"I-waitfix-{ctr}",
                            engine=inst.engine,
                            sync_info=mybir.SyncInfo(on_wait=[w], on_update=[]),
                            bass_nofuse=True,
                        ))
                    si.on_wait = si.on_wait[:cap]
                new_insts.append(inst)
            bb.instructions[:] = new_insts


def _build_bass():
    import concourse.bass as bass
    import concourse.mybir as mybir
    import concourse.tile as tile

    f32 = mybir.dt.float32
    nc = bass.Bass()
    x0T_d = nc.dram_tensor("x0T", [S, BS], f32, kind="ExternalInput")
    measT_d = nc.dram_tensor("measT", [8 * T, BS], f32, kind="ExternalInput")
    U_d = nc.dram_tensor("U", [S + 8 * CH, WCOLS], f32, kind="ExternalInput")
    Prep_d = nc.dram_tensor("Prep", [2, 128, 1600], f32, kind="ExternalInput")
    st_out = nc.dram_tensor("st", [2, 128, 3200], f32, kind="ExternalOutput")
    zp_out = nc.dram_tensor("zp", [2, 128, 1600], f32, kind="ExternalOutput")
    cv_out = nc.dram_tensor("cv", [BS // 8, 128, 3200], f32, kind="ExternalOutput")

    with tile.TileContext(nc) as tc:
        with tc.tile_pool(name="const", bufs=1) as cpool, \
             tc.tile_pool(name="stage", bufs=1) as spool, \
             tc.tile_pool(name="u", bufs=3) as upool, \
             tc.tile_pool(name="py", bufs=4, space="PSUM") as pypool, \
             tc.tile_pool(name="px", bufs=2, space="PSUM") as pxpool:

            # P-track source in two halves; first on the low-latency ACT
            # HWDGE ring so broadcast writes start ~2us in
            prs = [cpool.tile([128, 1600], f32, name=f"pr{i}") for i in (0, 1)]
            nc.scalar.dma_start(prs[0][:], Prep_d[0])
            nc.gpsimd.dma_start(prs[1][:], Prep_d[1])
            U = cpool.tile([S + 8 * CH, WCOLS], f32)
            nc.gpsimd.dma_start(U[:], U_d[:])

            # covariance broadcast: 2 DMAs per 8 batch rows (0.8 MB each)
            for i in range(BS // 8):
                nc.sync.dma_start(cv_out[i, :, 0:1600], prs[0][:])
                nc.sync.dma_start(cv_out[i, :, 1600:3200], prs[1][:])

            sts = [spool.tile([128, 3200], f32, tag=f"st{h}", name=f"st{h}")
                   for h in (0, 1)]
            zps = [spool.tile([128, 1600], f32, tag=f"zp{h}", name=f"zp{h}")
                   for h in (0, 1)]

            # chunk-0 input tiles: rows 0..15 = x0, rows 16.. = measurements
            u_cur = []
            for h in (0, 1):
                u0 = upool.tile([128, 128], f32, tag=f"u{h}", name=f"u0_{h}")
                nc.gpsimd.dma_start(u0[0:S, :], x0T_d[:, 128 * h:128 * h + 128])
                nc.gpsimd.dma_start(u0[S:S + 8 * CH, :],
                                    measT_d[0:8 * CH, 128 * h:128 * h + 128])
                u_cur.append(u0)

            for ch in range(NCH):
                c = CH if ch < NCH - 1 else LAST
                col0, kz = 24 * CH * ch, 8 * c
                K = S + kz
                for h in (0, 1):
                    u = u_cur[h]
                    py = pypool.tile([128, 24 * c], f32, tag="py", name=f"py{ch}_{h}")
                    nc.tensor.matmul(py[:], u[:K, :], U[:K, col0:col0 + 24 * c],
                                     start=True, stop=True)
                    if ch < NCH - 1:
                        c2 = CH if ch + 1 < NCH - 1 else LAST
                        un = upool.tile([128, 128], f32, tag=f"u{h}",
                                        name=f"u{ch + 1}_{h}")
                        nc.gpsimd.dma_start(
                            un[S:S + 8 * c2, :],
                            measT_d[8 * CH * (ch + 1):8 * CH * (ch + 1) + 8 * c2,
                                    128 * h:128 * h + 128])
                        ecol = col0 + 16 * c - 16
                        px = pxpool.tile([S, 128], f32, tag="px", name=f"px{ch}_{h}")
                        nc.tensor.matmul(px[:], U[:K, ecol:ecol + 16], u[:K, :],
                                         start=True, stop=True)
                        nc.vector.tensor_copy(un[0:S, :], px[:])
                        u_cur[h] = un
                    nc.vector.tensor_copy(sts[h][:, 16 * CH * ch:16 * CH * ch + 16 * c],
                                          py[:, :16 * c])
                    nc.vector.tensor_copy(zps[h][:, 8 * CH * ch:8 * CH * ch + 8 * c],
                                          py[:, 16 * c:24 * c])

            for h in (0, 1):
                nc.scalar.dma_start(st_out[h], sts[h][:])
                nc.scalar.dma_start(zp_out[h], zps[h][:])

    _fix_drain_waits(nc, mybir)
    return nc


def _ensure_axon_hooks():
    """This image's `antenv` lacks `axon_hooks`, which run_bass_kernel_spmd
    imports unconditionally when tracing is requested.  Provide a shim; if
    the libaxon profiling symbols exist, wire up the real ctypes hook."""
    import sys
    import types
    try:
        import antenv.axon_hooks  # noqa: F401
        return
    except ImportError:
        pass
    mod = types.ModuleType("antenv.axon_hooks")
    holder = [None]
    mod.set_axon_ntff_profile_hook = lambda h: holder.__setitem__(0, h)
    mod.get_axon_ntff_profile_hook = lambda: holder[0]
    sys.modules["antenv.axon_hooks"] = mod
    try:
        import antenv
        antenv.axon_hooks = mod
    except ImportError:
        pass
    try:
        from trn_agent_boot.trn_boot import _ntff_profile_via_ctypes
        hook = _ntff_profile_via_ctypes("/opt/axon/libaxon_pjrt.so")
        if hook is not None:
            mod.set_axon_ntff_profile_hook(hook)
    except Exception:
        pass


# ------------------------------------------------------------ numpy fallback

def _fallback(initial_state, initial_covariance, measurements, A, C,
              log_diag_Q, offdiag_Q, log_diag_R, offdiag_R):
    """Full per-batch EKF on host (used only if initial_covariance is not
    batch-constant, which setup_inputs never produces)."""
    Q = _learned_cov(log_diag_Q, offdiag_Q, S)
    R = _learned_cov(log_diag_R, offdiag_R, M)
    A = np.asarray(A, np.float64)
    C = np.asarray(C, np.float64)
    x = np.asarray(initial_state, np.float64)
    P = np.asarray(initial_covariance, np.float64)
    z = np.asarray(measurements, np.float64)
    I = np.eye(S)

    def regb(Pb):
        Pb = 0.5 * (Pb + np.swapaxes(Pb, -1, -2))
        return Pb + _EPS * I

    xs = np.empty((B, T, S), np.float32)
    Ps = np.empty((B, T, S, S), np.float32)
    zs = np.empty((B, T, M), np.float32)
    for t in range(T):
        x = x @ A.T
        P = regb(np.einsum('ij,bjk,lk->bil', A, P, A) + Q)
        zp = x @ C.T
        Sm = regb(np.einsum('ij,bjk,lk->bil', C, P, C) + R)
        Kg = np.einsum('bij,kj->bik', P, C) @ np.linalg.inv(Sm)
        x = x + np.einsum('bsm,bm->bs', Kg, z[:, t] - zp)
        IKH = I - Kg @ C
        P = regb(np.einsum('bij,bjk,blk->bil', IKH, P, IKH)
                 + np.einsum('bim,mn,bjn->bij', Kg, R, Kg))
        xs[:, t] = x
        Ps[:, t] = P
        zs[:, t] = zp
    return xs, Ps, zs


# ------------------------------------------------------------------- kernel

def kernel(initial_state, initial_covariance, measurements, A, C,
           log_diag_Q, offdiag_Q, log_diag_R, offdiag_R):
    global LAST_RESULT
    initial_state = np.asarray(initial_state, np.float32)
    initial_covariance = np.asarray(initial_covariance, np.float32)
    measurements = np.asarray(measurements, np.float32)

    P0 = initial_covariance[0]
    if initial_covariance.shape != (B, S, S) or \
       not (initial_covariance == P0[None]).all():
        return _fallback(initial_state, initial_covariance, measurements, A, C,
                         log_diag_Q, offdiag_Q, log_diag_R, offdiag_R)

    Q = _learned_cov(log_diag_Q, offdiag_Q, S)
    R = _learned_cov(log_diag_R, offdiag_R, M)
    Ms, Ns, Pseq, G = _p_track(A, C, Q, R, P0)
    U = _chunk_weights(Ms, Ns, G)
    Pflat = np.ascontiguousarray(Pseq, np.float32).reshape(T * S * S)
    Prep128 = np.tile(Pflat.reshape(16, 3200), (8, 1))   # [128, 3200]
    Prep = np.ascontiguousarray(
        np.stack([Prep128[:, :1600], Prep128[:, 1600:]]))  # [2, 128, 1600]

    x0T = np.ascontiguousarray(initial_state.T)                     # [S, B]
    measT = np.ascontiguousarray(measurements.reshape(B, 8 * T).T)  # [8T, B]

    if "nc" not in _CACHE:
        _CACHE["nc"] = _build_bass()
    nc = _CACHE["nc"]

    _ensure_axon_hooks()
    from concourse.bass_utils import run_bass_kernel_spmd
    in_maps = []
    for k in range(NCORES):
        bsl = slice(k * BS, (k + 1) * BS)
        in_maps.append({
            "x0T": np.ascontiguousarray(x0T[:, bsl]),
            "measT": np.ascontiguousarray(measT[:, bsl]),
            "U": U, "Prep": Prep,
        })
    res = run_bass_kernel_spmd(nc, in_maps, list(range(NCORES)))
    LAST_RESULT = res

    states = np.empty((B, T, S), np.float32)
    covs = np.empty((B, T, S, S), np.float32)
    zpred = np.empty((B, T, M), np.float32)
    for k in range(NCORES):
        bsl = slice(k * BS, (k + 1) * BS)
        r = res.results[k]
        states[bsl] = r["st"].reshape(BS, T, S)
        covs[bsl] = r["cv"].reshape(BS, T, S, S)
        zpred[bsl] = r["zp"].reshape(BS, T, M)
    return states, covs, zpred


# revision 24
# speedup vs baseline: 1.1775x; 1.1632x over previous
"""Trainium2 Bass kernel for the batched differentiable EKF.

Problem shape (hardcoded): B=2048, T=200, S=16 state dim, M=8 meas dim.

Structure exploited:
  * The covariance recursion (P_pred, S, K, P_new) never depends on the
    measurements — only on A, C, Q, R and the initial covariance.
  * The provided initial_covariance is identical for every batch element
    (checked at runtime), so the whole P/K track is batch-constant and is
    computed once on host in float64 (200 steps of 16x16 ops).
  * What remains per batch element is a linear time-varying recurrence
        x_t = x_{t-1} @ M_t + z_t @ N_t,   zpred_t = x_{t-1} @ G
    which is evaluated on device in chunks of 16 steps: each chunk is a
    single pair of accumulating matmuls against precomputed chunk-transfer
    weights, so the serial dependency chain is only 13 hops long.
  * The covariances output [B,T,S,S] (419 MB) is a broadcast of the
    batch-constant P track; the device replicates it to DRAM with wide
    SBUF->DRAM DMAs (8 batch rows = 1.6 MB per DMA).

Sharding: pure data parallel over batch, 256 rows per core, 8 cores.
"""

import numpy as np

_EPS = 1e-6      # numerical_stability_eps
_MIN_EIG = 1e-6  # LearnableCovariance.min_eigenvalue

B, T, S, M = 2048, 200, 16, 8
NCORES = 8
BS = B // NCORES          # 256 batch rows per core
CH = 14                   # chunk length (16 + 8*CH = 128 = exact K tile)
NCH = (T + CH - 1) // CH  # 15 chunks
LAST = T - CH * (NCH - 1) # 4 steps in the last chunk
WCOLS = 24 * T            # 4800 total weight columns (16+8 outputs/step)

_CACHE = {}
LAST_RESULT = None  # BassKernelResults of the most recent device run


# ---------------------------------------------------------------- host math

def _learned_cov(log_diag, off_diag, n):
    d = np.maximum(np.exp(np.asarray(log_diag, np.float64)), _MIN_EIG)
    L = np.diag(d)
    r, c = np.tril_indices(n, -1)
    if len(r) > 0:
        L[r, c] = np.asarray(off_diag, np.float64)
    return L @ L.T


def _reg(P):
    P = 0.5 * (P + P.T)
    return P + _EPS * np.eye(P.shape[-1], dtype=P.dtype)


def _p_track(A, C, Q, R, P0):
    """Batch-constant covariance recursion. Returns per-step state transfer
    M_t [S,S], measurement gain N_t [M,S], stored covariance P_t [S,S]."""
    A = np.asarray(A, np.float64)
    C = np.asarray(C, np.float64)
    P = np.asarray(P0, np.float64)
    I = np.eye(S)
    Ms = np.empty((T, S, S))
    Ns = np.empty((T, M, S))
    Pseq = np.empty((T, S, S))
    for t in range(T):
        Pp = _reg(A @ P @ A.T + Q)
        Sm = _reg(C @ Pp @ C.T + R)
        K = Pp @ C.T @ np.linalg.inv(Sm)
        IKH = I - K @ C
        Pn = _reg(IKH @ Pp @ IKH.T + K @ R @ K.T)
        Ms[t] = A.T @ IKH.T
        Ns[t] = K.T
        Pseq[t] = Pn
        P = Pn
    G = A.T @ C.T  # zpred_t = x_{t-1} @ G
    return Ms, Ns, Pseq, G


def _chunk_weights(Ms, Ns, G):
    """Per-chunk transfer weights mapping u = [x_chunk_start; z_1..z_c]
    (as rows) to [states(16c) | zpreds(8c)] (as columns).  Returned as a
    single [16+8*CH, WCOLS] array; chunk ch occupies columns
    24*CH*ch .. +24c, with rows 0..15 the x part and 16..16+8c the z part."""
    U = np.zeros((S + 8 * CH, WCOLS))
    for ch in range(NCH):
        c = CH if ch < NCH - 1 else LAST
        col0 = 24 * CH * ch
        V = np.zeros((S + 8 * c, S))
        V[:S, :S] = np.eye(S)
        for j in range(1, c + 1):
            t = CH * ch + j  # 1-indexed global step
            zp = V @ G       # zpred_t = X_{t-1} @ G
            czp = col0 + 16 * c + 8 * (j - 1)
            U[:S + 8 * c, czp:czp + 8] = zp
            V = V @ Ms[t - 1]
            V[S + 8 * (j - 1):S + 8 * j, :] += Ns[t - 1]
            cst = col0 + 16 * (j - 1)
            U[:S + 8 * c, cst:cst + 16] = V
    return np.ascontiguousarray(U, np.float32)


# ------------------------------------------------------------ device kernel

def _fix_drain_waits(nc, mybir):
    """This walrus build rejects instructions carrying more semaphore waits
    than their ctrl struct holds ("Too many sync wait commands") — seen on
    InstDrain and on matmul (waits migrate to LDWEIGHTS).  Cap inline waits
    (0 for Drain/Matmult, 1 otherwise); hoist the rest onto NoOps."""
    ctr = 0
    for f in nc.m.functions:
        for bb in f.blocks:
            new_insts = []
            for inst in bb.instructions:
                si = getattr(inst, "sync_info", None)
                cap = 1
                if isinstance(inst, (mybir.InstDrain, mybir.InstMatmult)):
                    cap = 0
                if si is not None and si.on_wait and len(si.on_wait) > cap:
                    for w in si.on_wait[cap:]:
                        ctr += 1
                        new_insts.append(mybir.InstNoOp(
                            name=f"I-waitfix-{ctr}",
                            engine=inst.engine,
                            sync_info=mybir.SyncInfo(on_wait=[w], on_update=[]),
                            bass_nofuse=True,
                        ))
                    si.on_wait = si.on_wait[:cap]
                new_insts.append(inst)
            bb.instructions[:] = new_insts


def _build_bass():
    import concourse.bass as bass
    import concourse.mybir as mybir
    import concourse.tile as tile

    f32 = mybir.dt.float32
    nc = bass.Bass()
    x0T_d = nc.dram_tensor("x0T", [S, BS], f32, kind="ExternalInput")
    measT_d = nc.dram_tensor("measT", [8 * T, BS], f32, kind="ExternalInput")
    U_d = nc.dram_tensor("U", [S + 8 * CH, WCOLS], f32, kind="ExternalInput")
    Prep_d = nc.dram_tensor("Prep", [2, 128, 1600], f32, kind="ExternalInput")
    st_out = nc.dram_tensor("st", [2, 128, 3200], f32, kind="ExternalOutput")
    zp_out = nc.dram_tensor("zp", [2, 128, 1600], f32, kind="ExternalOutput")
    cv_out = nc.dram_tensor("cv", [BS // 8, 128, 3200], f32, kind="ExternalOutput")

    with tile.TileContext(nc) as tc:
        with tc.tile_pool(name="const", bufs=1) as cpool, \
             tc.tile_pool(name="stage", bufs=1) as spool, \
             tc.tile_pool(name="u", bufs=3) as upool, \
             tc.tile_pool(name="py", bufs=4, space="PSUM") as pypool, \
             tc.tile_pool(name="px", bufs=2, space="PSUM") as pxpool:

            # P-track source in two halves; first on the low-latency ACT
            # HWDGE ring so broadcast writes start ~2us in
            prs = [cpool.tile([128, 1600], f32, name=f"pr{i}") for i in (0, 1)]
            nc.scalar.dma_start(prs[0][:], Prep_d[0])
            nc.gpsimd.dma_start(prs[1][:], Prep_d[1])
            U = cpool.tile([S + 8 * CH, WCOLS], f32)
            nc.gpsimd.dma_start(U[:], U_d[:])

            # covariance broadcast: 2 DMAs per 8 batch rows (0.8 MB each)
            for i in range(BS // 8):
                nc.sync.dma_start(cv_out[i, :, 0:1600], prs[0][:])
                nc.sync.dma_start(cv_out[i, :, 1600:3200], prs[1][:])

            sts = [spool.tile([128, 3200], f32, tag=f"st{h}", name=f"st{h}")
                   for h in (0, 1)]
            zps = [spool.tile([128, 1600], f32, tag=f"zp{h}", name=f"zp{h}")
                   for h in (0, 1)]

            # chunk-0 input tiles: rows 0..15 = x0, rows 16.. = measurements
            u_cur = []
            for h in (0, 1):
                u0 = upool.tile([128, 128], f32, tag=f"u{h}", name=f"u0_{h}")
                nc.gpsimd.dma_start(u0[0:S, :], x0T_d[:, 128 * h:128 * h + 128])
                nc.gpsimd.dma_start(u0[S:S + 8 * CH, :],
                                    measT_d[0:8 * CH, 128 * h:128 * h + 128])
                u_cur.append(u0)

            for ch in range(NCH):
                c = CH if ch < NCH - 1 else LAST
                col0, kz = 24 * CH * ch, 8 * c
                K = S + kz
                for h in (0, 1):
                    u = u_cur[h]
                    py = pypool.tile([128, 24 * c], f32, tag="py", name=f"py{ch}_{h}")
                    nc.tensor.matmul(py[:], u[:K, :], U[:K, col0:col0 + 24 * c],
                                     start=True, stop=True)
                    if ch < NCH - 1:
                        c2 = CH if ch + 1 < NCH - 1 else LAST
                        un = upool.tile([128, 128], f32, tag=f"u{h}",
                                        name=f"u{ch + 1}_{h}")
                        nc.gpsimd.dma_start(
                            un[S:S + 8 * c2, :],
                            measT_d[8 * CH * (ch + 1):8 * CH * (ch + 1) + 8 * c2,
                                    128 * h:128 * h + 128])
                        ecol = col0 + 16 * c - 16
                        px = pxpool.tile([S, 128], f32, tag="px", name=f"px{ch}_{h}")
                        nc.tensor.matmul(px[:], U[:K, ecol:ecol + 16], u[:K, :],
                                         start=True, stop=True)
                        nc.vector.tensor_copy(un[0:S, :], px[:])
                        u_cur[h] = un
                    nc.vector.tensor_copy(sts[h][:, 16 * CH * ch:16 * CH * ch + 16 * c],
                                          py[:, :16 * c])
                    nc.vector.tensor_copy(zps[h][:, 8 * CH * ch:8 * CH * ch + 8 * c],
                                          py[:, 16 * c:24 * c])

            for h in (0, 1):
                nc.scalar.dma_start(st_out[h], sts[h][:])
                nc.scalar.dma_start(zp_out[h], zps[h][:])

    _fix_drain_waits(nc, mybir)
    return nc


def _ensure_axon_hooks():
    """This image's `antenv` lacks `axon_hooks`, which run_bass_kernel_spmd
    imports unconditionally when tracing is requested.  Provide a shim; if
    the libaxon profiling symbols exist, wire up the real ctypes hook."""
    import sys
    import types
    try:
        import antenv.axon_hooks  # noqa: F401
        return
    except ImportError:
        pass
    mod = types.ModuleType("antenv.axon_hooks")
    holder = [None]
    mod.set_axon_ntff_profile_hook = lambda h: holder.__setitem__(0, h)
    mod.get_axon_ntff_profile_hook = lambda: holder[0]
    sys.modules["antenv.axon_hooks"] = mod
    try:
        import antenv
        antenv.axon_hooks = mod
    except ImportError:
        pass
    try:
        from trn_agent_boot.trn_boot import _ntff_profile_via_ctypes
        hook = _ntff_profile_via_ctypes("/opt/axon/libaxon_pjrt.so")
        if hook is not None:
            mod.set_axon_ntff_profile_hook(hook)
    except Exception:
        pass


def _run_spmd_cached(nc, in_maps):
    """Like bass2jax.run_bass_via_pjrt (multi-core branch) but the jitted
    executable is cached in _CACHE so repeat kernel() calls skip re-tracing
    and recompiling.  Returns a list of per-core {name: np.ndarray}."""
    import jax
    import numpy as _np
    import concourse.mybir as mybir
    from concourse import bass2jax

    if "exec" not in _CACHE:
        bass2jax.install_neuronx_cc_hook()
        partition_name = (nc.partition_id_tensor.name
                          if nc.partition_id_tensor else None)
        in_names, out_names, out_avals, zero_shapes = [], [], [], []
        for alloc in nc.m.functions[0].allocations:
            if not isinstance(alloc, mybir.MemoryLocationSet):
                continue
            name = alloc.memorylocations[0].name
            if alloc.kind == "ExternalInput":
                if name != partition_name:
                    in_names.append(name)
            elif alloc.kind == "ExternalOutput":
                shape = tuple(alloc.tensor_shape)
                dtype = mybir.dt.np(alloc.dtype)
                out_names.append(name)
                out_avals.append(jax.core.ShapedArray(shape, dtype))
                zero_shapes.append((shape, dtype))
        n_params = len(in_names)
        all_in_names = list(in_names) + list(out_names)
        if partition_name is not None:
            all_in_names.append(partition_name)

        def _body(*args):
            operands = list(args)
            if partition_name is not None:
                operands.append(bass2jax.partition_id_tensor())
            return tuple(bass2jax._bass_exec_p.bind(
                *operands,
                out_avals=tuple(out_avals),
                in_names=tuple(all_in_names),
                out_names=tuple(out_names),
                lowering_input_output_aliases=(),
                sim_require_finite=True,
                sim_require_nnan=True,
                nc=nc,
            ))

        devices = jax.devices()[:NCORES]
        mesh = bass2jax.Mesh(_np.asarray(devices), ("core",))
        n_outs = len(out_names)
        in_specs = (bass2jax.PartitionSpec("core"),) * (n_params + n_outs)
        out_specs = (bass2jax.PartitionSpec("core"),) * n_outs
        sharded = jax.jit(
            bass2jax.shard_map(_body, mesh=mesh, in_specs=in_specs,
                               out_specs=out_specs, check_rep=False),
            donate_argnums=tuple(range(n_params, n_params + n_outs)),
            keep_unused=True,
        )
        _CACHE["exec"] = (sharded, in_names, out_names, out_avals, zero_shapes)

    sharded, in_names, out_names, out_avals, zero_shapes = _CACHE["exec"]
    concat_in = [
        np.concatenate([np.asarray(in_maps[c][name]) for c in range(NCORES)],
                       axis=0)
        for name in in_names
    ]
    concat_zeros = [np.zeros((NCORES * s[0], *s[1:]), dt)
                    for s, dt in zero_shapes]
    out_arrs = sharded(*concat_in, *concat_zeros)
    return [
        {name: np.asarray(out_arrs[i]).reshape(NCORES, *out_avals[i].shape)[c]
         for i, name in enumerate(out_names)}
        for c in range(NCORES)
    ]


# ------------------------------------------------------------ numpy fallback

def _fallback(initial_state, initial_covariance, measurements, A, C,
              log_diag_Q, offdiag_Q, log_diag_R, offdiag_R):
    """Full per-batch EKF on host (used only if initial_covariance is not
    batch-constant, which setup_inputs never produces)."""
    Q = _learned_cov(log_diag_Q, offdiag_Q, S)
    R = _learned_cov(log_diag_R, offdiag_R, M)
    A = np.asarray(A, np.float64)
    C = np.asarray(C, np.float64)
    x = np.asarray(initial_state, np.float64)
    P = np.asarray(initial_covariance, np.float64)
    z = np.asarray(measurements, np.float64)
    I = np.eye(S)

    def regb(Pb):
        Pb = 0.5 * (Pb + np.swapaxes(Pb, -1, -2))
        return Pb + _EPS * np.eye(Pb.shape[-1])

    xs = np.empty((B, T, S), np.float32)
    Ps = np.empty((B, T, S, S), np.float32)
    zs = np.empty((B, T, M), np.float32)
    for t in range(T):
        x = x @ A.T
        P = regb(np.einsum('ij,bjk,lk->bil', A, P, A) + Q)
        zp = x @ C.T
        Sm = regb(np.einsum('ij,bjk,lk->bil', C, P, C) + R)
        Kg = np.einsum('bij,kj->bik', P, C) @ np.linalg.inv(Sm)
        x = x + np.einsum('bsm,bm->bs', Kg, z[:, t] - zp)
        IKH = I - Kg @ C
        P = regb(np.einsum('bij,bjk,blk->bil', IKH, P, IKH)
                 + np.einsum('bim,mn,bjn->bij', Kg, R, Kg))
        xs[:, t] = x
        Ps[:, t] = P
        zs[:, t] = zp
    return xs, Ps, zs


# ------------------------------------------------------------------- kernel

def kernel(initial_state, initial_covariance, measurements, A, C,
           log_diag_Q, offdiag_Q, log_diag_R, offdiag_R):
    global LAST_RESULT
    initial_state = np.asarray(initial_state, np.float32)
    initial_covariance = np.asarray(initial_covariance, np.float32)
    measurements = np.asarray(measurements, np.float32)

    P0 = initial_covariance[0]
    if initial_covariance.shape != (B, S, S) or \
       not (initial_covariance == P0[None]).all():
        return _fallback(initial_state, initial_covariance, measurements, A, C,
                         log_diag_Q, offdiag_Q, log_diag_R, offdiag_R)

    Q = _learned_cov(log_diag_Q, offdiag_Q, S)
    R = _learned_cov(log_diag_R, offdiag_R, M)
    Ms, Ns, Pseq, G = _p_track(A, C, Q, R, P0)
    U = _chunk_weights(Ms, Ns, G)
    Pflat = np.ascontiguousarray(Pseq, np.float32).reshape(T * S * S)
    Prep128 = np.tile(Pflat.reshape(16, 3200), (8, 1))   # [128, 3200]
    Prep = np.ascontiguousarray(
        np.stack([Prep128[:, :1600], Prep128[:, 1600:]]))  # [2, 128, 1600]

    x0T = np.ascontiguousarray(initial_state.T)                     # [S, B]
    measT = np.ascontiguousarray(measurements.reshape(B, 8 * T).T)  # [8T, B]

    if "nc" not in _CACHE:
        _CACHE["nc"] = _build_bass()
    nc = _CACHE["nc"]

    in_maps = []
    for k in range(NCORES):
        bsl = slice(k * BS, (k + 1) * BS)
        in_maps.append({
            "x0T": np.ascontiguousarray(x0T[:, bsl]),
            "measT": np.ascontiguousarray(measT[:, bsl]),
            "U": U, "Prep": Prep,
        })

    import os
    if os.environ.get("BASS_TRACE"):
        # profiling path (test harness): full run_bass_kernel_spmd with NTFF
        _ensure_axon_hooks()
        import concourse.bass_utils as _bu
        _bu.upload_artifacts = lambda tmpdir: f"local://{tmpdir}"
        res = _bu.run_bass_kernel_spmd(nc, in_maps, list(range(NCORES)))
        LAST_RESULT = res
        results = res.results
    else:
        results = _run_spmd_cached(nc, in_maps)

    states = np.empty((B, T, S), np.float32)
    covs = np.empty((B, T, S, S), np.float32)
    zpred = np.empty((B, T, M), np.float32)
    for k in range(NCORES):
        bsl = slice(k * BS, (k + 1) * BS)
        r = results[k]
        states[bsl] = r["st"].reshape(BS, T, S)
        covs[bsl] = r["cv"].reshape(BS, T, S, S)
        zpred[bsl] = r["zp"].reshape(BS, T, M)
    return states, covs, zpred
